# revision 19
# baseline (speedup 1.0000x reference)
"""ComPosHGNN Trainium2 kernel: 4-relation heterogeneous GraphConv.

Sharding: each relation's edges are bucketed by DESTINATION range (5000
nodes/core, 8 cores) -- every core computes its own slice of both output
node types, so no collectives are needed.  Host work is layout only
(bucket/sort/pad); all arithmetic (degrees, normalization, projection,
aggregation, relu) runs on device.

Per core pipeline:
  1. deg_out/deg_in from host-padded weight layouts (chunk-uniform pad
     widths) via reduce_sum, then rsqrt (clamped at eps).
  2. Scale each source table by rsqrt(deg_out) on the Activation engine
     into a bf16 HBM copy laid out partition-major (slot(n) =
     (n%128)*313 + n//128) so the writes are 4KB-contiguous per
     partition; gather indices are sigma-permuted to match.
  3. Per 5-tile dst group: one dma_gather per (relation, src-half)
     fetches the group's 128-edge blocks (bf16 rows); per block a
     w-scaled one-hot (DVE iota-compare) feeds a TensorE matmul
     accumulating the TRANSPOSED aggregate G^T[d, dst] in PSUM.
  4. Per-tile epilogue: G^T -> bf16, y = (G^T)^T @ W via a single
     matmul (no transpose needed), y*rsqrt(deg_in)+b, relu, and the
     two relations per output ntype are averaged and stored once.
"""
import numpy as np
import ml_dtypes
from contextlib import ExitStack

N_COM = 40000
N_POS = 40000
D = 128
NCORES = 8
SLICE = N_COM // NCORES          # 5000 dst nodes per core
TILES = 40                       # 39 full 128-row dst tiles + 1 partial (8 rows)
NT_TAB = 313                     # src table tiles (40064 = 313*128 padded rows)
NPAD = NT_TAB * 128
SHALF = 20032                    # sigma-space half split for int16 gather idx
KGRP = 4                         # dst tiles per gather group
NGRP = TILES // KGRP
EPS = 1e-20

# relation -> (src table, dst ntype); pairs share dst ntype
RELS = [
    ("demand", "com", "pos"),
    ("pflow", "pos", "pos"),
    ("supply", "pos", "com"),
    ("cflow", "com", "com"),
]
PAIRS = [("pos", ("demand", "pflow")), ("com", ("supply", "cflow"))]
TBL_RELS = {"com": ("demand", "cflow"), "pos": ("pflow", "supply")}


def _wrap_idx16(idx):
    """dma_gather index layout: idx i at [i%16, i//16], tiled x8 (Q7 cores)."""
    assert len(idx) % 16 == 0
    m = idx.astype(np.int16).reshape(-1, 16).T
    return np.tile(m, (8, 1))


def _dout_chunks():
    """(t0, nt) chunks over the 313 table tiles for deg_out padding."""
    ch = [(i * 16, 16) for i in range(19)]
    ch.append((304, 9))
    return ch


def _prep_relation(src, dst, w):
    """Host-side layout for one relation (all cores)."""
    src = np.asarray(src, np.int64)
    dst = np.asarray(dst, np.int64)
    w = np.asarray(w, np.float32)

    # --- deg_out pad (global, shared by cores): chunk-uniform widths ---
    counts_s = np.bincount(src, minlength=NPAD)
    chunks = _dout_chunks()
    pcs, offs, tot = [], [], 0
    for t0, nt in chunks:
        cmax = int(counts_s[t0 * 128:(t0 + nt) * 128].max())
        pc = max(8, ((cmax + 7) // 8) * 8)
        pcs.append(pc)
        offs.append(tot)
        tot += nt * pc
    dout_pad = np.zeros((128, tot), ml_dtypes.bfloat16)
    order_s = np.argsort(src, kind="stable")
    ssrc, sw = src[order_s], w[order_s]
    starts = np.zeros(NPAD, np.int64)
    starts[1:] = np.cumsum(counts_s)[:-1]
    rank = np.arange(len(ssrc)) - starts[ssrc]
    t_of = ssrc // 128
    p_of = ssrc % 128
    cidx = t_of // 16            # chunk id (313//16 -> last chunk is 9 tiles)
    cidx = np.minimum(cidx, len(chunks) - 1)
    pc_arr = np.asarray(pcs)[cidx]
    off_arr = np.asarray(offs)[cidx]
    t0_arr = np.asarray([c[0] for c in chunks])[cidx]
    col = off_arr + (t_of - t0_arr) * pc_arr + rank
    dout_pad[p_of, col] = sw.astype(ml_dtypes.bfloat16)

    # --- per-core edge bucketing ---
    core_of = dst // SLICE
    dloc_all = dst - core_of * SLICE
    tile_all = dloc_all // 128
    sig = (src % 128) * NT_TAB + src // 128      # sigma slot in scaled table
    half_all = sig // SHALF
    sloc_all = sig - half_all * SHALF

    counts_grid = np.zeros((NCORES, TILES, 2), np.int64)
    for k in range(NCORES):
        m = core_of == k
        np.add.at(counts_grid[k], (tile_all[m], half_all[m]), 1)
    bg = np.maximum((np.ceil(counts_grid.max(axis=0) / 128)).astype(np.int64), 1)
    NB = int(bg.sum())

    # block order: for g (tile groups), for h, for t in g
    okeys = []
    for g in range(NGRP):
        for h in range(2):
            for tt in range(KGRP):
                okeys.append((g * KGRP + tt) * 2 + h)
    pos_of_key = np.zeros(TILES * 2, np.int64)
    for i, kk in enumerate(okeys):
        pos_of_key[kk] = i
    # block start offset (in blocks) for each ordered (t,h)
    blk_at = np.zeros(TILES * 2 + 1, np.int64)
    acc = 0
    for i, kk in enumerate(okeys):
        t, h = kk // 2, kk % 2
        blk_at[i] = acc
        acc += int(bg[t, h])
    blk_at[TILES * 2] = acc
    assert acc == NB

    P_in = 8
    percore_masks = []
    for k in range(NCORES):
        m = core_of == k
        percore_masks.append(m)
        cnt_in = np.bincount(dloc_all[m], minlength=5120)
        P_in = max(P_in, ((int(cnt_in.max()) + 7) // 8) * 8)

    per_core = []
    for k in range(NCORES):
        m = percore_masks[k]
        s_k, w_k = sloc_all[m], w[m]
        t_k, h_k, dl_k = tile_all[m], half_all[m], dloc_all[m]

        cnt_in = np.bincount(dl_k, minlength=5120)
        deg_in_pad = np.zeros((5120, P_in), ml_dtypes.bfloat16)
        order_d = np.argsort(dl_k, kind="stable")
        sdl, swk = dl_k[order_d], w_k[order_d]
        st = np.zeros(5120, np.int64)
        st[1:] = np.cumsum(cnt_in)[:-1]
        deg_in_pad[sdl, np.arange(len(sdl)) - st[sdl]] = swk.astype(ml_dtypes.bfloat16)
        deg_in_cols = deg_in_pad.reshape(TILES, 128, P_in).transpose(1, 0, 2).reshape(
            128, TILES * P_in)

        gidx = np.zeros(NB * 128, np.int64)
        wcol = np.zeros(NB * 128, np.float32)
        dcol = np.zeros(NB * 128, np.float32)
        okey_e = pos_of_key[t_k * 2 + h_k]
        order = np.argsort(okey_e, kind="stable")
        s_o, w_o, d_o = s_k[order], w_k[order], dl_k[order]
        t_o = t_k[order]
        ko = okey_e[order]
        starts_g = np.searchsorted(ko, np.arange(TILES * 2))
        ends_g = np.searchsorted(ko, np.arange(TILES * 2) + 1)
        for i in range(TILES * 2):
            a, b = starts_g[i], ends_g[i]
            if a == b:
                continue
            o0 = blk_at[i] * 128
            n = b - a
            gidx[o0:o0 + n] = s_o[a:b]
            wcol[o0:o0 + n] = w_o[a:b]
            dcol[o0:o0 + n] = d_o[a:b] - t_o[a:b] * 128
        per_core.append({
            "gidx": _wrap_idx16(gidx),
            "wcol": wcol.reshape(NB, 128).T.copy(),
            "dcol": dcol.reshape(NB, 128).T.copy(),
            "deg_in": deg_in_cols,
        })
    meta = {"bg": bg, "NB": NB, "P_in": P_in, "pcs": pcs,
            "dout_tot": tot, "blk_at": blk_at, "okeys": okeys}
    return per_core, dout_pad, meta


def _build_kernel(shapes):
    import concourse.bass as bass  # noqa: F401
    import concourse.tile as tile
    from concourse import bacc, mybir

    f32 = mybir.dt.float32
    bf16 = mybir.dt.bfloat16
    nc = bacc.Bacc("TRN2", target_bir_lowering=False, debug=False,
                   enable_asserts=False, num_devices=NCORES)

    tabs = {
        "com": nc.dram_tensor("com_emb", [N_COM, D], f32, kind="ExternalInput"),
        "pos": nc.dram_tensor("pos_emb", [N_POS, D], f32, kind="ExternalInput"),
    }
    ins, scratch = {}, {}
    for rname, s_t, d_t in RELS:
        sh = shapes[rname]
        NB = sh["NB"]
        ins[rname] = {
            "gidx": nc.dram_tensor(f"{rname}_gidx", [128, NB * 8], mybir.dt.int16,
                                   kind="ExternalInput"),
            "wcol": nc.dram_tensor(f"{rname}_wcol", [128, NB], f32, kind="ExternalInput"),
            "dcol": nc.dram_tensor(f"{rname}_dcol", [128, NB], f32, kind="ExternalInput"),
            "dout": nc.dram_tensor(f"{rname}_degout", [128, sh["dout_tot"]], bf16,
                                   kind="ExternalInput"),
            "din": nc.dram_tensor(f"{rname}_degin", [128, TILES * sh["P_in"]], bf16,
                                  kind="ExternalInput"),
            "W": nc.dram_tensor(f"W_{rname}", [D, D], f32, kind="ExternalInput"),
            "b": nc.dram_tensor(f"b_{rname}", [1, D], f32, kind="ExternalInput"),
        }
        scratch[rname] = nc.dram_tensor(f"{rname}_scaled", [NPAD, D], bf16)
    out = nc.dram_tensor("out", [2, SLICE, D], f32, kind="ExternalOutput")

    chunks = _dout_chunks()
    # max gather-group block count over (rel, g, h)
    MAXGH = 0
    for rname, _, _ in RELS:
        bg = shapes[rname]["bg"]
        for g in range(NGRP):
            for h in range(2):
                MAXGH = max(MAXGH, int(bg[g * KGRP:(g + 1) * KGRP, h].sum()))

    with tile.TileContext(nc) as tc:
        with ExitStack() as ctx:
            const_p = ctx.enter_context(tc.tile_pool(name="const", bufs=1))
            keep = ctx.enter_context(tc.tile_pool(name="keep", bufs=1))

            iota_i = const_p.tile([128, 128], mybir.dt.int32)
            nc.gpsimd.iota(iota_i[:], pattern=[[1, 128]], base=0, channel_multiplier=0)
            iota_b = const_p.tile([128, 128], bf16)
            nc.vector.tensor_copy(iota_b[:], iota_i[:])

            rout = {}
            rin = {}
            W_bf = {}
            b_rep = {}

            # --- phase 1: degrees + weights, all relations ---
            with tc.tile_pool(name="deg", bufs=2) as deg_p:
                for rname, s_t, d_t in RELS:
                    sh = shapes[rname]
                    inr = ins[rname]
                    ro = keep.tile([128, NT_TAB], f32, tag=f"rout_{rname}",
                                   name=f"rout_{rname}")
                    rout[rname] = ro
                    do_t = deg_p.tile([128, sh["dout_tot"]], bf16, tag="dout")
                    nc.sync.dma_start(do_t[:], inr["dout"].ap())
                    for (t0, nt), pc, off in zip(chunks, sh["pcs"], sh["offs"]):
                        dv = do_t[:, off:off + nt * pc].rearrange(
                            "p (t q) -> p t q", q=pc)
                        nc.vector.reduce_sum(ro[:, t0:t0 + nt], dv,
                                             axis=mybir.AxisListType.X)
                    nc.vector.tensor_scalar_max(ro[:], ro[:], EPS)
                    nc.scalar.activation(ro[:], ro[:],
                                         mybir.ActivationFunctionType.Sqrt)
                    nc.vector.reciprocal(ro[:], ro[:])

                    P_in = sh["P_in"]
                    di_t = deg_p.tile([128, TILES * P_in], bf16, tag="din")
                    nc.sync.dma_start(di_t[:], inr["din"].ap())
                    ri = keep.tile([128, TILES], f32, tag=f"rin_{rname}",
                                   name=f"rin_{rname}")
                    rin[rname] = ri
                    nc.vector.reduce_sum(
                        ri[:], di_t[:].rearrange("p (t q) -> p t q", q=P_in),
                        axis=mybir.AxisListType.X)
                    nc.vector.tensor_scalar_max(ri[:], ri[:], EPS)
                    nc.scalar.activation(ri[:], ri[:],
                                         mybir.ActivationFunctionType.Sqrt)
                    nc.vector.reciprocal(ri[:], ri[:])

                    W_sb = const_p.tile([128, D], f32, tag=f"W_{rname}")
                    nc.sync.dma_start(W_sb[:], inr["W"].ap())
                    wb = const_p.tile([128, D], bf16, tag=f"Wb_{rname}")
                    nc.vector.tensor_copy(wb[:], W_sb[:])
                    W_bf[rname] = wb
                    b_row = const_p.tile([1, D], f32, tag=f"b_{rname}")
                    nc.sync.dma_start(b_row[:], inr["b"].ap())
                    br = const_p.tile([128, D], f32, tag=f"brep_{rname}")
                    nc.gpsimd.partition_broadcast(br[:], b_row[:])
                    b_rep[rname] = br

            # --- phase 2: scaled bf16 source-table copies (sigma layout) ---
            CT = 16  # tiles per scale chunk
            with tc.tile_pool(name="scl", bufs=3) as sclp:
                for tname in ("com", "pos"):
                    raw = tabs[tname]
                    src_v = raw.ap()[0:312 * 128, :].rearrange(
                        "(t p) d -> p t d", p=128)
                    dst_vs = {r: scratch[r].ap().rearrange(
                        "(p t) d -> p t d", t=NT_TAB) for r in TBL_RELS[tname]}
                    for t0 in range(0, 312, CT):
                        nt = min(CT, 312 - t0)
                        bt = sclp.tile([128, CT * D], f32, tag="raw")
                        nc.sync.dma_start(
                            bt[:, 0:nt * D].rearrange("p (t d) -> p t d", d=D),
                            src_v[:, t0:t0 + nt, :])
                        # first relation of the pair gates the gather phase:
                        # scale it on the Activation engine; the other on DVE
                        # so both drain concurrently.
                        for ri, r in enumerate(TBL_RELS[tname]):
                            sc = sclp.tile([128, CT * D], bf16, tag=f"sc_{r}")
                            for j in range(nt):
                                if ri == 0:
                                    nc.scalar.activation(
                                        sc[:, j * D:(j + 1) * D],
                                        bt[:, j * D:(j + 1) * D],
                                        mybir.ActivationFunctionType.Copy,
                                        scale=rout[r][:, t0 + j:t0 + j + 1])
                                else:
                                    nc.vector.tensor_scalar_mul(
                                        sc[:, j * D:(j + 1) * D],
                                        bt[:, j * D:(j + 1) * D],
                                        rout[r][:, t0 + j:t0 + j + 1])
                            nc.sync.dma_start(
                                dst_vs[r][:, t0:t0 + nt, :],
                                sc[:, 0:nt * D].rearrange("p (t d) -> p t d", d=D))
                    # partial tile 312 (64 rows)
                    lt = sclp.tile([128, D], f32, tag="raw_last")
                    nc.sync.dma_start(lt[0:64, :], raw.ap()[312 * 128:N_COM, :])
                    for ri, r in enumerate(TBL_RELS[tname]):
                        sl = sclp.tile([128, D], bf16, tag=f"scl_{r}")
                        if ri == 0:
                            nc.scalar.activation(
                                sl[0:64, :], lt[0:64, :],
                                mybir.ActivationFunctionType.Copy,
                                scale=rout[r][0:64, 312:313])
                        else:
                            nc.vector.tensor_scalar_mul(
                                sl[0:64, :], lt[0:64, :], rout[r][0:64, 312:313])
                        nc.sync.dma_start(dst_vs[r][0:64, 312:313, :],
                                          sl[0:64, :].rearrange("p (t d) -> p t d", d=D))

            # --- phase 3: per-pair gather + aggregate + epilogue ---
            MAXNB = max(shapes[r]["NB"] for r, _, _ in RELS)
            with tc.tile_pool(name="idx", bufs=1) as idxp, \
                 tc.tile_pool(name="g", bufs=2) as gp, \
                 tc.tile_pool(name="oh", bufs=4) as ohp, \
                 tc.tile_pool(name="ps", bufs=4, space="PSUM") as psp, \
                 tc.tile_pool(name="ps2", bufs=4, space="PSUM") as ps2, \
                 tc.tile_pool(name="ep", bufs=4) as ep:
                for ntype, pair in PAIRS:
                    oi = 0 if ntype == "com" else 1
                    edges = {}
                    for j, rname in enumerate(pair):
                        sh = shapes[rname]
                        NB = sh["NB"]
                        inr = ins[rname]
                        gi = idxp.tile([128, MAXNB * 8], mybir.dt.int16,
                                       tag=f"gidx_{j}")
                        nc.sync.dma_start(gi[:, 0:NB * 8], inr["gidx"].ap())
                        wc = idxp.tile([128, MAXNB], f32, tag=f"wcol_{j}")
                        nc.sync.dma_start(wc[:, 0:NB], inr["wcol"].ap())
                        dc = idxp.tile([128, MAXNB], f32, tag=f"dcol_{j}")
                        nc.sync.dma_start(dc[:, 0:NB], inr["dcol"].ap())
                        edges[rname] = (gi, wc, dc)

                    for g in range(NGRP):
                        gbufs = {}
                        for j, rname in enumerate(pair):
                            sh = shapes[rname]
                            bg = sh["bg"]
                            blk_at = sh["blk_at"]
                            gi, wc, dc = edges[rname]
                            halves = [scratch[rname].ap()[0:SHALF, :],
                                      scratch[rname].ap()[SHALF:NPAD, :]]
                            for h in range(2):
                                i0 = (g * 2 + h) * KGRP
                                b0 = int(blk_at[i0])
                                nbs = int(blk_at[i0 + KGRP] - b0)
                                ni = nbs * 128
                                gb = gp.tile([128, MAXGH * D], bf16,
                                             tag=f"g_{j}_{h}")
                                gv = gb[:].rearrange("p (b d) -> p b d", d=D)
                                nc.gpsimd.dma_gather(
                                    gv[:, 0:nbs, :], halves[h],
                                    gi[:, b0 * 8:(b0 + nbs) * 8],
                                    num_idxs=ni, num_idxs_reg=ni, elem_size=D,
                                    single_packet=False)
                                gbufs[(rname, h)] = (gb, b0)

                        for tt in range(KGRP):
                            t = g * KGRP + tt
                            outt = None
                            for rname in pair:
                                sh = shapes[rname]
                                bg = sh["bg"]
                                blk_at = sh["blk_at"]
                                gi, wc, dc = edges[rname]
                                ps = psp.tile([128, D], f32, tag="acc")
                                first = True
                                for h in range(2):
                                    i0 = (g * 2 + h) * KGRP
                                    gb, b0 = gbufs[(rname, h)]
                                    gv = gb[:].rearrange("p (b d) -> p b d", d=D)
                                    tb0 = int(blk_at[i0 + tt])       # global blk
                                    nb = int(bg[t, h])
                                    for b in range(nb):
                                        col = tb0 + b
                                        lb = tb0 - int(blk_at[i0]) + b  # in gbuf
                                        oh = ohp.tile([128, 128], bf16, tag="oh")
                                        nc.vector.tensor_scalar(
                                            oh[:], iota_b[:],
                                            dc[:, col:col + 1], wc[:, col:col + 1],
                                            op0=mybir.AluOpType.is_equal,
                                            op1=mybir.AluOpType.mult)
                                        nc.tensor.matmul(
                                            ps[:], gv[:, lb, :], oh[:],
                                            start=first,
                                            stop=(h == 1 and b == nb - 1))
                                        first = False
                                # epilogue for (t, rname): ps = G^T [d, dst]
                                gT = ep.tile([128, D], bf16, tag="gT")
                                nc.vector.tensor_copy(gT[:], ps[:])
                                y_ps = ps2.tile([128, D], f32, tag="y")
                                nc.tensor.matmul(y_ps[:], gT[:], W_bf[rname][:],
                                                 start=True, stop=True)
                                y1 = ep.tile([128, D], f32, tag="y1")
                                nc.vector.tensor_scalar_mul(
                                    y1[:], y_ps[:], rin[rname][:, t:t + 1])
                                y2 = ep.tile([128, D], f32, tag="y2")
                                nc.vector.tensor_add(y2[:], y1[:], b_rep[rname][:])
                                if outt is None:
                                    outt = ep.tile([128, D], f32, tag="outt")
                                    nc.vector.tensor_scalar(
                                        outt[:], y2[:], 0.0, 0.5,
                                        op0=mybir.AluOpType.max,
                                        op1=mybir.AluOpType.mult)
                                else:
                                    y3 = ep.tile([128, D], f32, tag="y3")
                                    nc.vector.tensor_scalar(
                                        y3[:], y2[:], 0.0, 0.5,
                                        op0=mybir.AluOpType.max,
                                        op1=mybir.AluOpType.mult)
                                    nc.vector.tensor_add(outt[:], outt[:], y3[:])
                            if t < 39:
                                nc.sync.dma_start(
                                    out.ap()[oi, t * 128:(t + 1) * 128, :], outt[:])
                            else:
                                nc.sync.dma_start(
                                    out.ap()[oi, 39 * 128:SLICE, :], outt[0:8, :])
    nc.compile()
    return nc


def _shapes_meta(metas):
    shapes = {}
    for rname, _, _ in RELS:
        m = metas[rname]
        offs = []
        tot = 0
        for (t0, nt), pc in zip(_dout_chunks(), m["pcs"]):
            offs.append(tot)
            tot += nt * pc
        shapes[rname] = {"bg": m["bg"], "NB": m["NB"], "P_in": m["P_in"],
                         "pcs": m["pcs"], "offs": offs, "dout_tot": m["dout_tot"],
                         "blk_at": m["blk_at"]}
    return shapes


LAST_RESULTS = None


def kernel(**inputs):
    global LAST_RESULTS
    from concourse.bass_utils import run_bass_kernel_spmd

    com_emb = np.asarray(inputs["com_emb"], np.float32)
    pos_emb = np.asarray(inputs["pos_emb"], np.float32)

    percore_rel, douts, metas = {}, {}, {}
    for rname, s_t, d_t in RELS:
        per_core, dout_pad, meta = _prep_relation(
            inputs[f"{rname}_src"], inputs[f"{rname}_dst"], inputs[f"{rname}_w"])
        percore_rel[rname] = per_core
        douts[rname] = dout_pad
        metas[rname] = meta

    shapes = _shapes_meta(metas)
    nc = _build_kernel(shapes)

    in_maps = []
    for k in range(NCORES):
        m = {"com_emb": com_emb, "pos_emb": pos_emb}
        for rname, s_t, d_t in RELS:
            pc = percore_rel[rname][k]
            m[f"{rname}_gidx"] = pc["gidx"]
            m[f"{rname}_wcol"] = pc["wcol"]
            m[f"{rname}_dcol"] = pc["dcol"]
            m[f"{rname}_degout"] = douts[rname]
            m[f"{rname}_degin"] = pc["deg_in"]
            m[f"W_{rname}"] = np.asarray(inputs[f"W_{rname}"], np.float32)
            m[f"b_{rname}"] = np.asarray(inputs[f"b_{rname}"], np.float32).reshape(1, D)
        in_maps.append(m)

    res = run_bass_kernel_spmd(nc, in_maps, core_ids=list(range(NCORES)))
    LAST_RESULTS = res
    out = np.empty((2, N_COM, D), np.float32)
    for k in range(NCORES):
        o = res.results[k]["out"]
        out[0, k * SLICE:(k + 1) * SLICE] = o[0]
        out[1, k * SLICE:(k + 1) * SLICE] = o[1]
    return out
